# revision 1
# baseline (speedup 1.0000x reference)
"""Block-sparse self-attention (DeepSpeed "fixed" layout) on 8 trn2 cores.

Problem: B=2, H=16, S=2048, D=64 fp32. Mask (identical for every head,
numverts=1): each 64-wide diagonal window is dense, plus every 4th
16-col block ("stripe") is attended by all queries. Per 64-row query
window the attended key set = 512 stripe cols + 48 non-stripe window
cols.

Sharding: 32 (b,h) pairs -> 4 per core (batch+head parallel).

Host prep per pair (pure layout + dtype cast; reorder puts the 512
stripe cols first, then 32 windows x 48 non-stripe cols):
  qT  [64, 2048]  Q^T
  kT  [64, 2560]  K^T reordered: 512 stripe cols, then 16 window-pairs
                  of 128 cols each [48 even | 16 zero | 48 odd | 16 zero]
                  (zero padding keeps engine partition bases 32-aligned)
  vs  [128, 4*65] stripe V_aug in on-chip layout: partition r, chunk c
                  holds V[reorder[c*128+r]] ++ [1]  (ones col -> softmax
                  denominator L rides the PV matmul)
  vw2 [112, 16*65] window-pair V_aug: partitions 0:48 = window 2i,
                  64:112 = window 2i+1 (48:64 zero), ones col each

On chip per pair (fp16 operands, software-pipelined across pairs so the
tensor engine never idles: pair p's QK fills interleave with pair p-1's
PV accumulation, drains split across ACT and DVE):
  stripe scores  S^T[k,q] = matmul(kt chunk, qt)      16x [128,512]
  window scores  [112,128] blocks: windows (2j, 2j+1) stacked on the
                 output-partition axis share one matmul; the off-
                 diagonal cross-window entries are computed but thrown
                 away by the block-diagonal exp-write into a zeroed P
  P = exp(0.125 * S): ACT engine for 6/8 stripe tiles (exact), DVE
                 Schraudolph-in-fp16-bits for chunk 3 + windows (~31%
                 of P, ~1.5% rms elementwise); keeps both drain engines
                 under the warm-clock PE pair period
  O'^T[65,q] accumulates stripe chunks ([65,512] x16) and window pairs
                 ([65,128] x16); row 64 = L (ones columns of V_aug)
  out[p] = O' [65, 2048] as fp16 via DVE copy + 2 DMAs
Host: O = (O'[0:64] / O'[64])^T per pair.
"""

import numpy as np

B, H, S, D = 2, 16, 2048, 64
NPAIRS = B * H
NCORES = 8
P_PER_CORE = NPAIRS // NCORES  # 4
NCH = 4        # stripe k-chunks of 128
NW = S // 64   # 32 windows
SCALE = float(D) ** -0.5


def _reorder_idx():
    blocks = np.arange(S // 16)
    stripe = blocks[blocks % 4 == 3]
    rest = blocks[blocks % 4 != 3]
    cols = np.arange(S).reshape(-1, 16)
    return np.concatenate([cols[stripe].ravel(), cols[rest].ravel()])


_REORDER = _reorder_idx()

_CACHE = {}


def _build(dt_in_name="float16", npairs=P_PER_CORE):
    from contextlib import ExitStack
    import concourse.bacc as bacc
    import concourse.tile as tile
    from concourse import mybir

    dt_in = getattr(mybir.dt, dt_in_name)
    f32 = mybir.dt.float32
    i16 = mybir.dt.int16
    EXP = mybir.ActivationFunctionType.Exp
    MUL = mybir.AluOpType.mult
    ADD = mybir.AluOpType.add
    # Schraudolph exp in fp16 bit space: fp16_bits(exp(s*SCALE)) ~
    # s * (SCALE*1024*log2 e) + (15*1024 - 59.3). One DVE tensor_scalar
    # (fp32 PSUM -> int16 convert) per tile; the int16 buffer is the fp16
    # P tile by bitcast. ~1.5% rms elementwise, applied to ~30% of P.
    SCH_A = SCALE * 1024.0 / float(np.log(2.0))
    SCH_B = 15.0 * 1024.0 - 59.3

    nc = bacc.Bacc("TRN2", target_bir_lowering=False, debug=False,
                   num_devices=NCORES)
    KW = 512 + 64 * NW  # padded kT width (2560)
    VW = NCH * 65 + (NW // 2) * 65  # 260 + 1040
    # layout: [kt_stripe 512 | qt 2048 | kt_windows 2048]
    qkT = nc.dram_tensor("qkT", [npairs, 64, S + KW], dt_in,
                         kind="ExternalInput").ap()
    vin = nc.dram_tensor("vin", [npairs, 128, VW], dt_in,
                         kind="ExternalInput").ap()
    out = nc.dram_tensor("out", [npairs, 65, S], dt_in,
                         kind="ExternalOutput").ap()

    with tile.TileContext(nc) as tc, ExitStack() as ctx:
        qk_pool = ctx.enter_context(tc.tile_pool(name="qk", bufs=2))
        v_pool = ctx.enter_context(tc.tile_pool(name="v", bufs=2))
        p_pool = ctx.enter_context(tc.tile_pool(name="p", bufs=2))
        s_pool = ctx.enter_context(tc.tile_pool(name="s", bufs=3, space="PSUM"))
        o_pool = ctx.enter_context(tc.tile_pool(name="o", bufs=2, space="PSUM"))

        # P-window tiles are persistent: the zero cross-window blocks are
        # zeroed once and stay zero (every pair overwrites only the same
        # diagonal blocks), so no per-pair memset is needed.
        pw_tiles = [p_pool.tile([112, S], dt_in, tag=f"pw{b}",
                                name=f"pw_{b}") for b in range(2)]

        def load_tiles(p):
            qk = qk_pool.tile([64, S + KW], dt_in, tag="qk")
            # stripe-K + first q-half first (all that rounds 0-3 need),
            # the rest behind it on a second queue. For pair 0 the first
            # block is split across two queues (Scalar is idle then) to
            # pull the very first fill earlier.
            if p == 0:
                nc.sync.dma_start(out=qk[:, 0:768], in_=qkT[p, :, 0:768])
                nc.scalar.dma_start(out=qk[:, 768:1536],
                                    in_=qkT[p, :, 768:1536])
            else:
                nc.sync.dma_start(out=qk[:, 0:1536],
                                  in_=qkT[p, :, 0:1536])
            nc.gpsimd.dma_start(out=qk[:, 1536:2560],
                                in_=qkT[p, :, 1536:2560])
            nc.gpsimd.dma_start(out=qk[:, 2560:4608],
                                in_=qkT[p, :, 2560:4608])
            vt = v_pool.tile([128, VW], dt_in, tag="v")
            nc.sync.dma_start(out=vt, in_=vin[p])
            ps = p_pool.tile([128, NCH, S], dt_in, tag="ps")
            return dict(p=p, kt=qk[:, 0:512], qt=qk[:, 512:2560],
                        ktw=qk[:, 2560:4608],
                        vst=vt[:, 0:NCH * 65],
                        vwt=vt[0:112, NCH * 65:VW], ps=ps,
                        pw=pw_tiles[p % 2])

        def pv_step(cx, i):
            # i in 0..31: per q-quarter qg: 4 stripe MMs then 4 window MMs.
            # O'^T accumulates in a [65, 512] quarter; V_aug ones col lands
            # the softmax denominator L in row 64.
            qg, r = i // 8, i % 8
            if r == 0:
                cx["ov" + str(qg)] = o_pool.tile([128, 512], f32, tag="o",
                                                 name=f"ov{cx['p']}_{qg}")
            ov = cx["ov" + str(qg)]
            if r < 4:
                c = r
                nc.tensor.matmul(
                    out=ov[0:65, :],
                    lhsT=cx["vst"][:, c * 65:(c + 1) * 65],
                    rhs=cx["ps"][:, c, qg * 512:(qg + 1) * 512],
                    start=(c == 0), stop=False, skip_group_check=True)
            else:
                j = 4 * qg + (r - 4)
                q0 = (j % 4) * 128
                nc.tensor.matmul(
                    out=ov[0:65, q0:q0 + 128],
                    lhsT=cx["vwt"][:, j * 65:(j + 1) * 65],
                    rhs=cx["pw"][0:112, 128 * j:128 * j + 128],
                    start=False, stop=(r == 7), skip_group_check=True)

        def pv_copy(cx, qg):
            # PSUM -> SBUF staging (DMA cannot read PSUM), all on DVE,
            # fp32 -> fp16 in the copy; out-DMA per half on two queues
            if qg == 0:
                cx["ob"] = p_pool.tile([65, S], dt_in, tag="ob",
                                       name=f"ob{cx['p']}")
            ob = cx["ob"]
            ov = cx["ov" + str(qg)]
            nc.vector.tensor_copy(ob[:, qg * 512:(qg + 1) * 512], ov[0:65, :])
            if qg == 1:
                nc.sync.dma_start(out=out[cx["p"], :, 0:1024],
                                  in_=ob[:, 0:1024])
            if qg == 3:
                nc.gpsimd.dma_start(out=out[cx["p"], :, 1024:2048],
                                    in_=ob[:, 1024:2048])

        ctxs = [load_tiles(0)]
        for pwb in pw_tiles:
            nc.gpsimd.memset(pwb, 0.0)
        for p in range(npairs):
            nxt_needed = p + 1 < npairs
            cur = ctxs[p]
            prev = ctxs[p - 1] if p > 0 else None
            # emit, prefetching next pair's tiles after the first round
            qt, kt, ps, pw = cur["qt"], cur["kt"], cur["ps"], cur["pw"]
            for r in range(8):
                c, g = r % 4, r // 4
                st = s_pool.tile([128, 1024], f32, tag="s")
                for u in range(2):
                    q0 = g * 1024 + u * 512
                    nc.tensor.matmul(
                        out=st[:, u * 512:(u + 1) * 512],
                        lhsT=kt[:, c * 128:(c + 1) * 128],
                        rhs=qt[:, q0:q0 + 512],
                        start=True, stop=True)
                po = ps[:, c, g * 1024:(g + 1) * 1024]
                if c == 3:
                    nc.vector.tensor_scalar(
                        out=po.bitcast(i16), in0=st,
                        scalar1=SCH_A, scalar2=SCH_B, op0=MUL, op1=ADD)
                else:
                    nc.scalar.activation(out=po, in_=st,
                                         func=EXP, scale=SCALE)
                if r == 0 and nxt_needed:
                    ctxs.append(load_tiles(p + 1))
                if prev is not None:
                    for i in range(4 * r, 4 * r + 4):
                        pv_step(prev, i)
                    if r % 2 == 1:
                        pv_copy(prev, r // 2)
            for h in range(2):
                sw = s_pool.tile([128, 1024], f32, tag="s")
                for j in range(8 * h, 8 * h + 8):
                    fo = (j - 8 * h) * 128
                    nc.tensor.matmul(
                        out=sw[0:112, fo:fo + 128],
                        lhsT=cur["ktw"][:, 128 * j:128 * j + 112],
                        rhs=qt[:, 128 * j:128 * j + 128],
                        start=True, stop=True)
                sw4 = sw.rearrange("p (j t f) -> p j t f", t=2, f=64)
                pw4 = pw.rearrange("p (j t f) -> p j t f", t=2, f=64)
                nc.vector.tensor_scalar(
                    out=pw4[0:48, 8 * h:8 * h + 8, 0, :].bitcast(i16),
                    in0=sw4[0:48, 0:8, 0, :],
                    scalar1=SCH_A, scalar2=SCH_B, op0=MUL, op1=ADD)
                nc.vector.tensor_scalar(
                    out=pw4[64:112, 8 * h:8 * h + 8, 1, :].bitcast(i16),
                    in0=sw4[64:112, 0:8, 1, :],
                    scalar1=SCH_A, scalar2=SCH_B, op0=MUL, op1=ADD)
        # flush: PV of the last pair
        last = ctxs[-1]
        for i in range(32):
            pv_step(last, i)
            if i % 8 == 7:
                pv_copy(last, i // 8)

    nc.compile()
    return nc


def _get_nc(dt_in_name="float16"):
    if dt_in_name not in _CACHE:
        _CACHE[dt_in_name] = _build(dt_in_name)
    return _CACHE[dt_in_name]


def _prep_inputs(query, key, value, np_dt):
    q = np.asarray(query).reshape(NPAIRS, S, D)
    k = np.asarray(key).reshape(NPAIRS, S, D)
    v = np.asarray(value).reshape(NPAIRS, S, D)
    kr = k[:, _REORDER, :]
    vr = v[:, _REORDER, :]
    # qkT: [stripe K^T 512 | Q^T 2048 | window K^T padded: pairs of
    # 128 cols as [48 even | 16 zero | 48 odd | 16 zero]]
    KW = 512 + 64 * NW
    qkT = np.zeros((NPAIRS, 64, S + KW), np_dt)
    kTs = kr.transpose(0, 2, 1).astype(np_dt)  # [P, 64, 2048]
    qkT[:, :, 0:512] = kTs[:, :, 0:512]
    qkT[:, :, 512:512 + S] = q.transpose(0, 2, 1)
    kw = kTs[:, :, 512:].reshape(NPAIRS, 64, NW // 2, 2, 48)
    kTw = qkT[:, :, 512 + S:].reshape(NPAIRS, 64, NW // 2, 2, 64)
    kTw[:, :, :, :, 0:48] = kw
    va = np.concatenate(
        [vr, np.ones((NPAIRS, S, 1), vr.dtype)], axis=2).astype(np_dt)
    # vin: stripe V_aug [partition r, chunk c, 65] ++ window-pair V_aug
    # [112 rows (0:48 window 2i, 64:112 window 2i+1, 48:64 zero), i, 65]
    VW = NCH * 65 + (NW // 2) * 65
    vin = np.zeros((NPAIRS, 128, VW), np_dt)
    vin[:, :, 0:NCH * 65] = (
        va[:, :512].reshape(NPAIRS, NCH, 128, 65).transpose(0, 2, 1, 3)
    ).reshape(NPAIRS, 128, NCH * 65)
    vw = va[:, 512:].reshape(NPAIRS, NW // 2, 2, 48, 65)
    vwin = vin[:, :, NCH * 65:].reshape(NPAIRS, 128, NW // 2, 65)
    vwin[:, 0:48] = vw[:, :, 0].transpose(0, 2, 1, 3)
    vwin[:, 64:112] = vw[:, :, 1].transpose(0, 2, 1, 3)
    in_maps = []
    for core in range(NCORES):
        sl = slice(core * P_PER_CORE, (core + 1) * P_PER_CORE)
        in_maps.append({"qkT": np.ascontiguousarray(qkT[sl]),
                        "vin": np.ascontiguousarray(vin[sl])})
    return in_maps


def _run(query, key, value, dt_in_name="float16", trace=False):
    from concourse.bass_utils import run_bass_kernel_spmd
    nc = _get_nc(dt_in_name)
    in_maps = _prep_inputs(query, key, value, np.float16
                           if dt_in_name == "float16" else np.float32)
    res = run_bass_kernel_spmd(nc, in_maps, list(range(NCORES)), trace=trace)
    o = np.concatenate([res.results[i]["out"] for i in range(NCORES)],
                       axis=0).astype(np.float32)
    full = (o[:, 0:64, :] / o[:, 64:65, :]).transpose(0, 2, 1).reshape(
        B, H, S, D).astype(np.float32)
    return full, res


def kernel(query, key, value):
    full, _ = _run(np.asarray(query), np.asarray(key), np.asarray(value))
    return full



# revision 3
# speedup vs baseline: 1.1734x; 1.1734x over previous
"""Block-sparse self-attention (DeepSpeed "fixed" layout) on 8 trn2 cores.

Problem: B=2, H=16, S=2048, D=64 fp32. Mask (identical for every head,
numverts=1): each 64-wide diagonal window is dense, plus every 4th
16-col block ("stripe") is attended by all queries. Per 64-row query
window the attended key set = 512 stripe cols + 48 non-stripe window
cols.

Sharding: 32 (b,h) pairs -> 4 per core (batch+head parallel).

Host prep per pair (pure layout + dtype cast; reorder puts the 512
stripe cols first, then 32 windows x 48 non-stripe cols):
  qT  [64, 2048]  Q^T
  kT  [64, 2560]  K^T reordered: 512 stripe cols, then 16 window-pairs
                  of 128 cols each [48 even | 16 zero | 48 odd | 16 zero]
                  (zero padding keeps engine partition bases 32-aligned)
  vs  [128, 4*65] stripe V_aug in on-chip layout: partition r, chunk c
                  holds V[reorder[c*128+r]] ++ [1]  (ones col -> softmax
                  denominator L rides the PV matmul)
  vw2 [112, 16*65] window-pair V_aug: partitions 0:48 = window 2i,
                  64:112 = window 2i+1 (48:64 zero), ones col each

On chip per pair (fp16 operands, software-pipelined across pairs so the
tensor engine never idles: pair p's QK fills interleave with pair p-1's
PV accumulation, drains split across ACT and DVE):
  stripe scores  S^T[k,q] = matmul(kt chunk, qt)      16x [128,512]
  window scores  [112,128] blocks: windows (2j, 2j+1) stacked on the
                 output-partition axis share one matmul; the off-
                 diagonal cross-window entries are computed but thrown
                 away by the block-diagonal exp-write into a zeroed P
  P = exp(0.125 * S): ACT engine for 6/8 stripe tiles (exact), DVE
                 Schraudolph-in-fp16-bits for chunk 3 + windows (~31%
                 of P, ~1.5% rms elementwise); keeps both drain engines
                 under the warm-clock PE pair period
  O'^T[65,q] accumulates stripe chunks ([65,512] x16) and window pairs
                 ([65,128] x16); row 64 = L (ones columns of V_aug)
  out[p] = O' [65, 2048] as fp16 via DVE copy + 2 DMAs
Host: O = (O'[0:64] / O'[64])^T per pair.
"""

import numpy as np

B, H, S, D = 2, 16, 2048, 64
NPAIRS = B * H
NCORES = 8
P_PER_CORE = NPAIRS // NCORES  # 4
NCH = 4        # stripe k-chunks of 128
NW = S // 64   # 32 windows
SCALE = float(D) ** -0.5


def _reorder_idx():
    blocks = np.arange(S // 16)
    stripe = blocks[blocks % 4 == 3]
    rest = blocks[blocks % 4 != 3]
    cols = np.arange(S).reshape(-1, 16)
    return np.concatenate([cols[stripe].ravel(), cols[rest].ravel()])


_REORDER = _reorder_idx()

_CACHE = {}


def _build(dt_in_name="float16", npairs=P_PER_CORE):
    from contextlib import ExitStack
    import concourse.bacc as bacc
    import concourse.tile as tile
    from concourse import mybir

    dt_in = getattr(mybir.dt, dt_in_name)
    f32 = mybir.dt.float32
    i16 = mybir.dt.int16
    EXP = mybir.ActivationFunctionType.Exp
    MUL = mybir.AluOpType.mult
    ADD = mybir.AluOpType.add
    # Schraudolph exp in fp16/bf16 bit space: bits(exp(s*SCALE)) ~
    # s * (SCALE*2^m*log2 e) + (bias*2^m - 0.0579*2^m). One DVE
    # tensor_scalar (fp32 PSUM -> int16 convert) per tile; the int16
    # buffer is the 16-bit-float P tile by bitcast. ~1.5% rms
    # elementwise, applied to ~30% of P.
    if dt_in_name == "float16":
        SCH_A = SCALE * 1024.0 / float(np.log(2.0))
        SCH_B = 15.0 * 1024.0 - 59.3
    else:  # bfloat16
        SCH_A = SCALE * 128.0 / float(np.log(2.0))
        SCH_B = 127.0 * 128.0 - 7.4

    nc = bacc.Bacc("TRN2", target_bir_lowering=False, debug=False,
                   num_devices=NCORES)
    KW = 512 + 64 * NW  # padded kT width (2560)
    VW = NCH * 65 + (NW // 2) * 65  # 260 + 1040
    # layout: [kt_stripe 512 | qt 2048 | kt_windows 2048]
    qkT = nc.dram_tensor("qkT", [npairs, 64, S + KW], dt_in,
                         kind="ExternalInput").ap()
    vin = nc.dram_tensor("vin", [npairs, 128, VW], dt_in,
                         kind="ExternalInput").ap()
    out = nc.dram_tensor("out", [npairs, 65, S], dt_in,
                         kind="ExternalOutput").ap()

    with tile.TileContext(nc) as tc, ExitStack() as ctx:
        qk_pool = ctx.enter_context(tc.tile_pool(name="qk", bufs=2))
        v_pool = ctx.enter_context(tc.tile_pool(name="v", bufs=2))
        p_pool = ctx.enter_context(tc.tile_pool(name="p", bufs=2))
        s_pool = ctx.enter_context(tc.tile_pool(name="s", bufs=3, space="PSUM"))
        o_pool = ctx.enter_context(tc.tile_pool(name="o", bufs=2, space="PSUM"))

        # P-window tiles are persistent: the zero cross-window blocks are
        # zeroed once and stay zero (every pair overwrites only the same
        # diagonal blocks), so no per-pair memset is needed.
        pw_tiles = [p_pool.tile([112, S], dt_in, tag=f"pw{b}",
                                name=f"pw_{b}") for b in range(2)]

        def load_tiles(p):
            qk = qk_pool.tile([64, S + KW], dt_in, tag="qk")
            # stripe-K + first q-half first (all that rounds 0-3 need),
            # the rest behind it on a second queue. For pair 0 the first
            # block is split across two queues (Scalar is idle then) to
            # pull the very first fill earlier.
            if p == 0:
                nc.sync.dma_start(out=qk[:, 0:768], in_=qkT[p, :, 0:768])
                nc.scalar.dma_start(out=qk[:, 768:1536],
                                    in_=qkT[p, :, 768:1536])
            else:
                nc.sync.dma_start(out=qk[:, 0:1536],
                                  in_=qkT[p, :, 0:1536])
            nc.gpsimd.dma_start(out=qk[:, 1536:2560],
                                in_=qkT[p, :, 1536:2560])
            nc.gpsimd.dma_start(out=qk[:, 2560:4608],
                                in_=qkT[p, :, 2560:4608])
            vt = v_pool.tile([128, VW], dt_in, tag="v")
            nc.sync.dma_start(out=vt, in_=vin[p])
            ps = p_pool.tile([128, NCH, S], dt_in, tag="ps")
            return dict(p=p, kt=qk[:, 0:512], qt=qk[:, 512:2560],
                        ktw=qk[:, 2560:4608],
                        vst=vt[:, 0:NCH * 65],
                        vwt=vt[0:112, NCH * 65:VW], ps=ps,
                        pw=pw_tiles[p % 2])

        def pv_step(cx, i):
            # i in 0..31: per q-quarter qg: 4 stripe MMs then 4 window MMs.
            # O'^T accumulates in a [65, 512] quarter; V_aug ones col lands
            # the softmax denominator L in row 64.
            qg, r = i // 8, i % 8
            if r == 0:
                cx["ov" + str(qg)] = o_pool.tile([128, 512], f32, tag="o",
                                                 name=f"ov{cx['p']}_{qg}")
            ov = cx["ov" + str(qg)]
            if r < 4:
                c = r
                nc.tensor.matmul(
                    out=ov[0:65, :],
                    lhsT=cx["vst"][:, c * 65:(c + 1) * 65],
                    rhs=cx["ps"][:, c, qg * 512:(qg + 1) * 512],
                    start=(c == 0), stop=False, skip_group_check=True)
            else:
                j = 4 * qg + (r - 4)
                q0 = (j % 4) * 128
                nc.tensor.matmul(
                    out=ov[0:65, q0:q0 + 128],
                    lhsT=cx["vwt"][:, j * 65:(j + 1) * 65],
                    rhs=cx["pw"][0:112, 128 * j:128 * j + 128],
                    start=False, stop=(r == 7), skip_group_check=True)

        def pv_copy(cx, qg):
            # PSUM -> SBUF staging (DMA cannot read PSUM), all on DVE,
            # fp32 -> fp16 in the copy; out-DMA per half on two queues
            if qg == 0:
                cx["ob"] = p_pool.tile([65, S], dt_in, tag="ob",
                                       name=f"ob{cx['p']}")
            ob = cx["ob"]
            ov = cx["ov" + str(qg)]
            nc.vector.tensor_copy(ob[:, qg * 512:(qg + 1) * 512], ov[0:65, :])
            if qg == 1:
                nc.sync.dma_start(out=out[cx["p"], :, 0:1024],
                                  in_=ob[:, 0:1024])
            if qg == 3:
                nc.gpsimd.dma_start(out=out[cx["p"], :, 1024:2048],
                                    in_=ob[:, 1024:2048])

        ctxs = [load_tiles(0)]
        for pwb in pw_tiles:
            nc.gpsimd.memset(pwb, 0.0)
        for p in range(npairs):
            nxt_needed = p + 1 < npairs
            cur = ctxs[p]
            prev = ctxs[p - 1] if p > 0 else None
            # emit, prefetching next pair's tiles after the first round
            qt, kt, ps, pw = cur["qt"], cur["kt"], cur["ps"], cur["pw"]
            for r in range(8):
                c, g = r % 4, r // 4
                st = s_pool.tile([128, 1024], f32, tag="s")
                for u in range(2):
                    q0 = g * 1024 + u * 512
                    nc.tensor.matmul(
                        out=st[:, u * 512:(u + 1) * 512],
                        lhsT=kt[:, c * 128:(c + 1) * 128],
                        rhs=qt[:, q0:q0 + 512],
                        start=True, stop=True)
                po = ps[:, c, g * 1024:(g + 1) * 1024]
                if c == 3:
                    nc.vector.tensor_scalar(
                        out=po.bitcast(i16), in0=st,
                        scalar1=SCH_A, scalar2=SCH_B, op0=MUL, op1=ADD)
                else:
                    nc.scalar.activation(out=po, in_=st,
                                         func=EXP, scale=SCALE)
                if r == 0 and nxt_needed:
                    ctxs.append(load_tiles(p + 1))
                if prev is not None:
                    for i in range(4 * r, 4 * r + 4):
                        pv_step(prev, i)
                    if r % 2 == 1:
                        pv_copy(prev, r // 2)
            for h in range(2):
                sw = s_pool.tile([128, 1024], f32, tag="s")
                for j in range(8 * h, 8 * h + 8):
                    fo = (j - 8 * h) * 128
                    nc.tensor.matmul(
                        out=sw[0:112, fo:fo + 128],
                        lhsT=cur["ktw"][:, 128 * j:128 * j + 112],
                        rhs=qt[:, 128 * j:128 * j + 128],
                        start=True, stop=True)
                sw4 = sw.rearrange("p (j t f) -> p j t f", t=2, f=64)
                pw4 = pw.rearrange("p (j t f) -> p j t f", t=2, f=64)
                nc.vector.tensor_scalar(
                    out=pw4[0:48, 8 * h:8 * h + 8, 0, :].bitcast(i16),
                    in0=sw4[0:48, 0:8, 0, :],
                    scalar1=SCH_A, scalar2=SCH_B, op0=MUL, op1=ADD)
                nc.vector.tensor_scalar(
                    out=pw4[64:112, 8 * h:8 * h + 8, 1, :].bitcast(i16),
                    in0=sw4[64:112, 0:8, 1, :],
                    scalar1=SCH_A, scalar2=SCH_B, op0=MUL, op1=ADD)
        # flush: PV of the last pair
        last = ctxs[-1]
        for i in range(32):
            pv_step(last, i)
            if i % 8 == 7:
                pv_copy(last, i // 8)

    nc.compile()
    return nc


def _get_nc(dt_in_name="float16"):
    if dt_in_name not in _CACHE:
        _CACHE[dt_in_name] = _build(dt_in_name)
    return _CACHE[dt_in_name]


def _prep_inputs(query, key, value, np_dt):
    q = np.asarray(query).reshape(NPAIRS, S, D)
    k = np.asarray(key).reshape(NPAIRS, S, D)
    v = np.asarray(value).reshape(NPAIRS, S, D)
    kr = k[:, _REORDER, :]
    vr = v[:, _REORDER, :]
    # qkT: [stripe K^T 512 | Q^T 2048 | window K^T padded: pairs of
    # 128 cols as [48 even | 16 zero | 48 odd | 16 zero]]
    KW = 512 + 64 * NW
    qkT = np.zeros((NPAIRS, 64, S + KW), np_dt)
    kTs = kr.transpose(0, 2, 1).astype(np_dt)  # [P, 64, 2048]
    qkT[:, :, 0:512] = kTs[:, :, 0:512]
    qkT[:, :, 512:512 + S] = q.transpose(0, 2, 1)
    kw = kTs[:, :, 512:].reshape(NPAIRS, 64, NW // 2, 2, 48)
    kTw = qkT[:, :, 512 + S:].reshape(NPAIRS, 64, NW // 2, 2, 64)
    kTw[:, :, :, :, 0:48] = kw
    va = np.concatenate(
        [vr, np.ones((NPAIRS, S, 1), vr.dtype)], axis=2).astype(np_dt)
    # vin: stripe V_aug [partition r, chunk c, 65] ++ window-pair V_aug
    # [112 rows (0:48 window 2i, 64:112 window 2i+1, 48:64 zero), i, 65]
    VW = NCH * 65 + (NW // 2) * 65
    vin = np.zeros((NPAIRS, 128, VW), np_dt)
    vin[:, :, 0:NCH * 65] = (
        va[:, :512].reshape(NPAIRS, NCH, 128, 65).transpose(0, 2, 1, 3)
    ).reshape(NPAIRS, 128, NCH * 65)
    vw = va[:, 512:].reshape(NPAIRS, NW // 2, 2, 48, 65)
    vwin = vin[:, :, NCH * 65:].reshape(NPAIRS, 128, NW // 2, 65)
    vwin[:, 0:48] = vw[:, :, 0].transpose(0, 2, 1, 3)
    vwin[:, 64:112] = vw[:, :, 1].transpose(0, 2, 1, 3)
    in_maps = []
    for core in range(NCORES):
        sl = slice(core * P_PER_CORE, (core + 1) * P_PER_CORE)
        in_maps.append({"qkT": np.ascontiguousarray(qkT[sl]),
                        "vin": np.ascontiguousarray(vin[sl])})
    return in_maps


def _np_dt(dt_in_name):
    if dt_in_name == "float16":
        return np.float16
    if dt_in_name == "bfloat16":
        import ml_dtypes
        return ml_dtypes.bfloat16
    return np.float32


def _run(query, key, value, dt_in_name="float16", trace=False):
    from concourse.bass_utils import run_bass_kernel_spmd
    nc = _get_nc(dt_in_name)
    in_maps = _prep_inputs(query, key, value, _np_dt(dt_in_name))
    res = run_bass_kernel_spmd(nc, in_maps, list(range(NCORES)), trace=trace)
    o = np.concatenate([res.results[i]["out"] for i in range(NCORES)],
                       axis=0).astype(np.float32)
    full = (o[:, 0:64, :] / o[:, 64:65, :]).transpose(0, 2, 1).reshape(
        B, H, S, D).astype(np.float32)
    return full, res


def kernel(query, key, value):
    full, _ = _run(np.asarray(query), np.asarray(key), np.asarray(value))
    return full



# revision 7
# speedup vs baseline: 1.2568x; 1.0710x over previous
"""Block-sparse self-attention (DeepSpeed "fixed" layout) on 8 trn2 cores.

Problem: B=2, H=16, S=2048, D=64 fp32. Mask (identical for every head,
numverts=1): each 64-wide diagonal window is dense, plus every 4th
16-col block ("stripe") is attended by all queries. Per 64-row query
window the attended key set = 512 stripe cols + 48 non-stripe window
cols.

Sharding: 32 (b,h) pairs -> 4 per core (batch+head parallel).

Host prep per pair (pure layout + dtype cast; reorder puts the 512
stripe cols first, then 32 windows x 48 non-stripe cols):
  qT  [64, 2048]  Q^T
  kT  [64, 2560]  K^T reordered: 512 stripe cols, then 16 window-pairs
                  of 128 cols each [48 even | 16 zero | 48 odd | 16 zero]
                  (zero padding keeps engine partition bases 32-aligned)
  vs  [128, 4*65] stripe V_aug in on-chip layout: partition r, chunk c
                  holds V[reorder[c*128+r]] ++ [1]  (ones col -> softmax
                  denominator L rides the PV matmul)
  vw2 [112, 16*65] window-pair V_aug: partitions 0:48 = window 2i,
                  64:112 = window 2i+1 (48:64 zero), ones col each

On chip per pair (fp16 operands, software-pipelined across pairs so the
tensor engine never idles: pair p's QK fills interleave with pair p-1's
PV accumulation, drains split across ACT and DVE):
  stripe scores  S^T[k,q] = matmul(kt chunk, qt)      16x [128,512]
  window scores  [112,128] blocks: windows (2j, 2j+1) stacked on the
                 output-partition axis share one matmul; the off-
                 diagonal cross-window entries are computed but thrown
                 away by the block-diagonal exp-write into a zeroed P
  P = exp(0.125 * S): ACT engine for 6/8 stripe tiles (exact), DVE
                 Schraudolph-in-fp16-bits for chunk 3 + windows (~31%
                 of P, ~1.5% rms elementwise); keeps both drain engines
                 under the warm-clock PE pair period
  O'^T[65,q] accumulates stripe chunks ([65,512] x16) and window pairs
                 ([65,128] x16); row 64 = L (ones columns of V_aug)
  out[p] = O' [65, 2048] as fp16 via DVE copy + 2 DMAs
Host: O = (O'[0:64] / O'[64])^T per pair.
"""

import numpy as np

B, H, S, D = 2, 16, 2048, 64
NPAIRS = B * H
NCORES = 8
P_PER_CORE = NPAIRS // NCORES  # 4
NCH = 4        # stripe k-chunks of 128
NW = S // 64   # 32 windows
SCALE = float(D) ** -0.5


def _reorder_idx():
    blocks = np.arange(S // 16)
    stripe = blocks[blocks % 4 == 3]
    rest = blocks[blocks % 4 != 3]
    cols = np.arange(S).reshape(-1, 16)
    return np.concatenate([cols[stripe].ravel(), cols[rest].ravel()])


_REORDER = _reorder_idx()

_CACHE = {}


def _build(dt_in_name="float16", npairs=P_PER_CORE):
    from contextlib import ExitStack
    import concourse.bacc as bacc
    import concourse.tile as tile
    from concourse import mybir

    dt_in = getattr(mybir.dt, dt_in_name)
    f32 = mybir.dt.float32
    i16 = mybir.dt.int16
    EXP = mybir.ActivationFunctionType.Exp
    MUL = mybir.AluOpType.mult
    ADD = mybir.AluOpType.add
    # Schraudolph exp in fp16/bf16 bit space: bits(exp(s*SCALE)) ~
    # s * (SCALE*2^m*log2 e) + (bias*2^m - 0.0579*2^m). One DVE
    # tensor_scalar (fp32 PSUM -> int16 convert) per tile; the int16
    # buffer is the 16-bit-float P tile by bitcast. ~1.5% rms
    # elementwise, applied to ~30% of P.
    if dt_in_name == "float16":
        SCH_A = SCALE * 1024.0 / float(np.log(2.0))
        SCH_B = 15.0 * 1024.0 - 59.3
    else:  # bfloat16
        SCH_A = SCALE * 128.0 / float(np.log(2.0))
        SCH_B = 127.0 * 128.0 - 7.4

    nc = bacc.Bacc("TRN2", target_bir_lowering=False, debug=False,
                   num_devices=NCORES)
    KW = 512 + 64 * NW  # padded kT width (2560)
    VW = NCH * 65 + (NW // 2) * 65  # 260 + 1040
    # layout: [kt_stripe 512 | qt 2048 | kt_windows 2048]
    qkT = nc.dram_tensor("qkT", [npairs, 64, S + KW], dt_in,
                         kind="ExternalInput").ap()
    vin = nc.dram_tensor("vin", [npairs, 128, VW], dt_in,
                         kind="ExternalInput").ap()
    out = nc.dram_tensor("out", [npairs, 65, S], dt_in,
                         kind="ExternalOutput").ap()

    with tile.TileContext(nc) as tc, ExitStack() as ctx:
        qk_pool = ctx.enter_context(tc.tile_pool(name="qk", bufs=2))
        v_pool = ctx.enter_context(tc.tile_pool(name="v", bufs=2))
        p_pool = ctx.enter_context(tc.tile_pool(name="p", bufs=2))
        s_pool = ctx.enter_context(tc.tile_pool(name="s", bufs=3, space="PSUM"))
        o_pool = ctx.enter_context(tc.tile_pool(name="o", bufs=2, space="PSUM"))

        # P-window tiles are persistent: the zero cross-window blocks are
        # zeroed once and stay zero (every pair overwrites only the same
        # diagonal blocks), so no per-pair memset is needed.
        pw_tiles = [p_pool.tile([112, S], dt_in, tag=f"pw{b}",
                                name=f"pw_{b}") for b in range(2)]

        def load_tiles(p):
            qk = qk_pool.tile([64, S + KW], dt_in, tag="qk")
            # stripe-K + first q-half first (all that rounds 0-3 need),
            # the rest behind it on a second queue. For pair 0 the first
            # block is split across many queues to pull the very first
            # fill as early as possible (PE idles until it lands).
            if p == 0:
                for qi in range(8):
                    c0 = qi * 128
                    eng = nc.sync if qi % 2 == 0 else nc.scalar
                    eng.dma_start(out=qk[:, c0:c0 + 128],
                                  in_=qkT[p, :, c0:c0 + 128])
                nc.sync.dma_start(out=qk[:, 1024:1536],
                                  in_=qkT[p, :, 1024:1536])
            else:
                nc.sync.dma_start(out=qk[:, 0:1536],
                                  in_=qkT[p, :, 0:1536])
            nc.gpsimd.dma_start(out=qk[:, 1536:2560],
                                in_=qkT[p, :, 1536:2560])
            nc.gpsimd.dma_start(out=qk[:, 2560:4608],
                                in_=qkT[p, :, 2560:4608])
            vt = v_pool.tile([128, VW], dt_in, tag="v")
            nc.sync.dma_start(out=vt, in_=vin[p])
            ps = p_pool.tile([128, NCH, S], dt_in, tag="ps")
            return dict(p=p, kt=qk[:, 0:512], qt=qk[:, 512:2560],
                        ktw=qk[:, 2560:4608],
                        vst=vt[:, 0:NCH * 65],
                        vwt=vt[0:112, NCH * 65:VW], ps=ps,
                        pw=pw_tiles[p % 2])

        def pv_step(cx, i):
            # i in 0..31: per q-quarter qg: 4 stripe MMs then 4 window MMs.
            # O'^T accumulates in a [65, 512] quarter; V_aug ones col lands
            # the softmax denominator L in row 64.
            qg, r = i // 8, i % 8
            if r == 0:
                cx["ov" + str(qg)] = o_pool.tile([128, 512], f32, tag="o",
                                                 name=f"ov{cx['p']}_{qg}")
            ov = cx["ov" + str(qg)]
            if r < 4:
                c = r
                nc.tensor.matmul(
                    out=ov[0:65, :],
                    lhsT=cx["vst"][:, c * 65:(c + 1) * 65],
                    rhs=cx["ps"][:, c, qg * 512:(qg + 1) * 512],
                    start=(c == 0), stop=False, skip_group_check=True)
            else:
                j = 4 * qg + (r - 4)
                q0 = (j % 4) * 128
                nc.tensor.matmul(
                    out=ov[0:65, q0:q0 + 128],
                    lhsT=cx["vwt"][:, j * 65:(j + 1) * 65],
                    rhs=cx["pw"][0:112, 128 * j:128 * j + 128],
                    start=False, stop=(r == 7), skip_group_check=True)

        def pv_copy(cx, qg):
            # PSUM -> SBUF staging (DMA cannot read PSUM), all on DVE,
            # fp32 -> fp16 in the copy; out-DMA per half on two queues
            if qg == 0:
                cx["ob"] = p_pool.tile([65, S], dt_in, tag="ob",
                                       name=f"ob{cx['p']}")
            ob = cx["ob"]
            ov = cx["ov" + str(qg)]
            nc.vector.tensor_copy(ob[:, qg * 512:(qg + 1) * 512], ov[0:65, :])
            if qg == 1:
                nc.sync.dma_start(out=out[cx["p"], :, 0:1024],
                                  in_=ob[:, 0:1024])
            if qg == 3:
                nc.gpsimd.dma_start(out=out[cx["p"], :, 1024:2048],
                                    in_=ob[:, 1024:2048])

        # PE clock warmup: the HAM throttles the PE array to half clock
        # until it has seen a few microseconds of sustained matmul
        # activity. Burn junk matmuls (one weight load, pure streaming)
        # into a scratch PSUM bank while the first DMAs are in flight so
        # the real QK matmuls start at the full 2.4 GHz clock.
        warm_sb = v_pool.tile([128, 576], dt_in, tag="warm", name="warm_sb")
        nc.gpsimd.memset(warm_sb, 0.0)
        warm_ps = o_pool.tile([128, 512], f32, tag="o", name="warm_ps")
        for _ in range(12):
            nc.tensor.matmul(out=warm_ps[0:64, :], lhsT=warm_sb[:, 0:64],
                             rhs=warm_sb[:, 64:576], start=True, stop=True,
                             skip_group_check=True)

        ctxs = [load_tiles(0)]
        for pwb in pw_tiles:
            nc.gpsimd.memset(pwb, 0.0)
        for p in range(npairs):
            nxt_needed = p + 1 < npairs
            cur = ctxs[p]
            prev = ctxs[p - 1] if p > 0 else None
            # emit, prefetching next pair's tiles after the first round
            qt, kt, ps, pw = cur["qt"], cur["kt"], cur["ps"], cur["pw"]
            for r in range(8):
                c, g = r % 4, r // 4
                st = s_pool.tile([128, 1024], f32, tag="s")
                for u in range(2):
                    q0 = g * 1024 + u * 512
                    nc.tensor.matmul(
                        out=st[:, u * 512:(u + 1) * 512],
                        lhsT=kt[:, c * 128:(c + 1) * 128],
                        rhs=qt[:, q0:q0 + 512],
                        start=True, stop=True)
                po = ps[:, c, g * 1024:(g + 1) * 1024]
                if c == 3:
                    nc.vector.tensor_scalar(
                        out=po.bitcast(i16), in0=st,
                        scalar1=SCH_A, scalar2=SCH_B, op0=MUL, op1=ADD)
                else:
                    nc.scalar.activation(out=po, in_=st,
                                         func=EXP, scale=SCALE)
                if r == 0 and nxt_needed:
                    ctxs.append(load_tiles(p + 1))
                if prev is not None:
                    for i in range(4 * r, 4 * r + 4):
                        pv_step(prev, i)
                    if r % 2 == 1:
                        pv_copy(prev, r // 2)
            for h in range(2):
                sw = s_pool.tile([128, 1024], f32, tag="s")
                for j in range(8 * h, 8 * h + 8):
                    fo = (j - 8 * h) * 128
                    nc.tensor.matmul(
                        out=sw[0:112, fo:fo + 128],
                        lhsT=cur["ktw"][:, 128 * j:128 * j + 112],
                        rhs=qt[:, 128 * j:128 * j + 128],
                        start=True, stop=True)
                sw4 = sw.rearrange("p (j t f) -> p j t f", t=2, f=64)
                pw4 = pw.rearrange("p (j t f) -> p j t f", t=2, f=64)
                nc.vector.tensor_scalar(
                    out=pw4[0:48, 8 * h:8 * h + 8, 0, :].bitcast(i16),
                    in0=sw4[0:48, 0:8, 0, :],
                    scalar1=SCH_A, scalar2=SCH_B, op0=MUL, op1=ADD)
                nc.vector.tensor_scalar(
                    out=pw4[64:112, 8 * h:8 * h + 8, 1, :].bitcast(i16),
                    in0=sw4[64:112, 0:8, 1, :],
                    scalar1=SCH_A, scalar2=SCH_B, op0=MUL, op1=ADD)
        # flush: PV of the last pair
        last = ctxs[-1]
        for i in range(32):
            pv_step(last, i)
            if i % 8 == 7:
                pv_copy(last, i // 8)

    nc.compile()
    return nc


def _get_nc(dt_in_name="float16"):
    if dt_in_name not in _CACHE:
        _CACHE[dt_in_name] = _build(dt_in_name)
    return _CACHE[dt_in_name]


def _prep_inputs(query, key, value, np_dt):
    q = np.asarray(query).reshape(NPAIRS, S, D)
    k = np.asarray(key).reshape(NPAIRS, S, D)
    v = np.asarray(value).reshape(NPAIRS, S, D)
    kr = k[:, _REORDER, :]
    vr = v[:, _REORDER, :]
    # qkT: [stripe K^T 512 | Q^T 2048 | window K^T padded: pairs of
    # 128 cols as [48 even | 16 zero | 48 odd | 16 zero]]
    KW = 512 + 64 * NW
    qkT = np.zeros((NPAIRS, 64, S + KW), np_dt)
    kTs = kr.transpose(0, 2, 1).astype(np_dt)  # [P, 64, 2048]
    qkT[:, :, 0:512] = kTs[:, :, 0:512]
    qkT[:, :, 512:512 + S] = q.transpose(0, 2, 1)
    kw = kTs[:, :, 512:].reshape(NPAIRS, 64, NW // 2, 2, 48)
    kTw = qkT[:, :, 512 + S:].reshape(NPAIRS, 64, NW // 2, 2, 64)
    kTw[:, :, :, :, 0:48] = kw
    va = np.concatenate(
        [vr, np.ones((NPAIRS, S, 1), vr.dtype)], axis=2).astype(np_dt)
    # vin: stripe V_aug [partition r, chunk c, 65] ++ window-pair V_aug
    # [112 rows (0:48 window 2i, 64:112 window 2i+1, 48:64 zero), i, 65]
    VW = NCH * 65 + (NW // 2) * 65
    vin = np.zeros((NPAIRS, 128, VW), np_dt)
    vin[:, :, 0:NCH * 65] = (
        va[:, :512].reshape(NPAIRS, NCH, 128, 65).transpose(0, 2, 1, 3)
    ).reshape(NPAIRS, 128, NCH * 65)
    vw = va[:, 512:].reshape(NPAIRS, NW // 2, 2, 48, 65)
    vwin = vin[:, :, NCH * 65:].reshape(NPAIRS, 128, NW // 2, 65)
    vwin[:, 0:48] = vw[:, :, 0].transpose(0, 2, 1, 3)
    vwin[:, 64:112] = vw[:, :, 1].transpose(0, 2, 1, 3)
    in_maps = []
    for core in range(NCORES):
        sl = slice(core * P_PER_CORE, (core + 1) * P_PER_CORE)
        in_maps.append({"qkT": np.ascontiguousarray(qkT[sl]),
                        "vin": np.ascontiguousarray(vin[sl])})
    return in_maps


def _np_dt(dt_in_name):
    if dt_in_name == "float16":
        return np.float16
    if dt_in_name == "bfloat16":
        import ml_dtypes
        return ml_dtypes.bfloat16
    return np.float32


def _run(query, key, value, dt_in_name="float16", trace=False):
    from concourse.bass_utils import run_bass_kernel_spmd
    nc = _get_nc(dt_in_name)
    in_maps = _prep_inputs(query, key, value, _np_dt(dt_in_name))
    res = run_bass_kernel_spmd(nc, in_maps, list(range(NCORES)), trace=trace)
    o = np.concatenate([res.results[i]["out"] for i in range(NCORES)],
                       axis=0).astype(np.float32)
    full = (o[:, 0:64, :] / o[:, 64:65, :]).transpose(0, 2, 1).reshape(
        B, H, S, D).astype(np.float32)
    return full, res


def kernel(query, key, value):
    full, _ = _run(np.asarray(query), np.asarray(key), np.asarray(value))
    return full



# revision 15
# speedup vs baseline: 1.2752x; 1.0147x over previous
"""Block-sparse self-attention (DeepSpeed "fixed" layout) on 8 trn2 cores.

Problem: B=2, H=16, S=2048, D=64 fp32. Mask (identical for every head,
numverts=1): each 64-wide diagonal window is dense, plus every 4th
16-col block ("stripe") is attended by all queries. Per 64-row query
window the attended key set = 512 stripe cols + 48 non-stripe window
cols.

Sharding: 32 (b,h) pairs -> 4 per core (batch+head parallel).

Host prep per pair (pure layout + dtype cast; reorder puts the 512
stripe cols first, then 32 windows x 48 non-stripe cols):
  qT  [64, 2048]  Q^T
  kT  [64, 2560]  K^T reordered: 512 stripe cols, then 16 window-pairs
                  of 128 cols each [48 even | 16 zero | 48 odd | 16 zero]
                  (zero padding keeps engine partition bases 32-aligned)
  vs  [128, 4*65] stripe V_aug in on-chip layout: partition r, chunk c
                  holds V[reorder[c*128+r]] ++ [1]  (ones col -> softmax
                  denominator L rides the PV matmul)
  vw2 [112, 16*65] window-pair V_aug: partitions 0:48 = window 2i,
                  64:112 = window 2i+1 (48:64 zero), ones col each

On chip per pair (fp16 operands, software-pipelined across pairs so the
tensor engine never idles: pair p's QK fills interleave with pair p-1's
PV accumulation, drains split across ACT and DVE):
  stripe scores  S^T[k,q] = matmul(kt chunk, qt)      16x [128,512]
  window scores  [112,128] blocks: windows (2j, 2j+1) stacked on the
                 output-partition axis share one matmul; the off-
                 diagonal cross-window entries are computed but thrown
                 away by the block-diagonal exp-write into a zeroed P
  P = exp(0.125 * S): ACT engine for 6/8 stripe tiles (exact), DVE
                 Schraudolph-in-fp16-bits for chunk 3 + windows (~31%
                 of P, ~1.5% rms elementwise); keeps both drain engines
                 under the warm-clock PE pair period
  O'^T[65,q] accumulates stripe chunks ([65,512] x16) and window pairs
                 ([65,128] x16); row 64 = L (ones columns of V_aug)
  out[p] = O' [65, 2048] as fp16 via DVE copy + 2 DMAs
Host: O = (O'[0:64] / O'[64])^T per pair.
"""

import numpy as np

B, H, S, D = 2, 16, 2048, 64
NPAIRS = B * H
NCORES = 8
P_PER_CORE = NPAIRS // NCORES  # 4
NCH = 4        # stripe k-chunks of 128
NW = S // 64   # 32 windows
SCALE = float(D) ** -0.5


def _reorder_idx():
    blocks = np.arange(S // 16)
    stripe = blocks[blocks % 4 == 3]
    rest = blocks[blocks % 4 != 3]
    cols = np.arange(S).reshape(-1, 16)
    return np.concatenate([cols[stripe].ravel(), cols[rest].ravel()])


_REORDER = _reorder_idx()

_CACHE = {}


def _build(dt_in_name="float16", npairs=P_PER_CORE):
    from contextlib import ExitStack
    import concourse.bacc as bacc
    import concourse.tile as tile
    from concourse import mybir

    dt_in = getattr(mybir.dt, dt_in_name)
    f32 = mybir.dt.float32
    i16 = mybir.dt.int16
    EXP = mybir.ActivationFunctionType.Exp
    MUL = mybir.AluOpType.mult
    ADD = mybir.AluOpType.add
    # Schraudolph exp in fp16/bf16 bit space: bits(exp(s*SCALE)) ~
    # s * (SCALE*2^m*log2 e) + (bias*2^m - 0.0579*2^m). One DVE
    # tensor_scalar (fp32 PSUM -> int16 convert) per tile; the int16
    # buffer is the 16-bit-float P tile by bitcast. ~1.5% rms
    # elementwise, applied to ~30% of P.
    if dt_in_name == "float16":
        SCH_A = SCALE * 1024.0 / float(np.log(2.0))
        SCH_B = 15.0 * 1024.0 - 59.3
    else:  # bfloat16
        SCH_A = SCALE * 128.0 / float(np.log(2.0))
        SCH_B = 127.0 * 128.0 - 7.4

    nc = bacc.Bacc("TRN2", target_bir_lowering=False, debug=False,
                   num_devices=NCORES)
    KW = 512 + 64 * NW  # padded kT width (2560)
    VW = NCH * 65 + (NW // 2) * 65  # 260 + 1040
    # layout: [kt_stripe 512 | qt 2048 | kt_windows 2048]
    qkT = nc.dram_tensor("qkT", [npairs, 64, S + KW], dt_in,
                         kind="ExternalInput").ap()
    vin = nc.dram_tensor("vin", [npairs, 128, VW], dt_in,
                         kind="ExternalInput").ap()
    out = nc.dram_tensor("out", [npairs, 65, S], dt_in,
                         kind="ExternalOutput").ap()

    with tile.TileContext(nc) as tc, ExitStack() as ctx:
        qk_pool = ctx.enter_context(tc.tile_pool(name="qk", bufs=2))
        v_pool = ctx.enter_context(tc.tile_pool(name="v", bufs=2))
        p_pool = ctx.enter_context(tc.tile_pool(name="p", bufs=2))
        s_pool = ctx.enter_context(tc.tile_pool(name="s", bufs=3, space="PSUM"))
        o_pool = ctx.enter_context(tc.tile_pool(name="o", bufs=2, space="PSUM"))

        # P-window tiles are persistent: the zero cross-window blocks are
        # zeroed once and stay zero (every pair overwrites only the same
        # diagonal blocks), so no per-pair memset is needed.
        pw_tiles = [p_pool.tile([112, S], dt_in, tag=f"pw{b}",
                                name=f"pw_{b}") for b in range(2)]

        def load_tiles(p):
            qk = qk_pool.tile([64, S + KW], dt_in, tag="qk")
            # stripe-K + first q-half first (all that rounds 0-3 need),
            # the rest behind it on a second queue. For pair 0 the first
            # block is split across many queues to pull the very first
            # fill as early as possible (PE idles until it lands).
            if p == 0:
                # one dma_start already fans out across all 16 SDMA
                # engines; two here only to overlap the ~0.6us fixed
                # completion latency of the first with the second.
                nc.sync.dma_start(out=qk[:, 0:1024], in_=qkT[p, :, 0:1024])
                nc.scalar.dma_start(out=qk[:, 1024:1536],
                                    in_=qkT[p, :, 1024:1536])
            else:
                nc.sync.dma_start(out=qk[:, 0:1536],
                                  in_=qkT[p, :, 0:1536])
            nc.gpsimd.dma_start(out=qk[:, 1536:2560],
                                in_=qkT[p, :, 1536:2560])
            nc.gpsimd.dma_start(out=qk[:, 2560:4608],
                                in_=qkT[p, :, 2560:4608])
            vt = v_pool.tile([128, VW], dt_in, tag="v")
            nc.sync.dma_start(out=vt, in_=vin[p])
            ps = p_pool.tile([128, NCH, S], dt_in, tag="ps")
            return dict(p=p, kt=qk[:, 0:512], qt=qk[:, 512:2560],
                        ktw=qk[:, 2560:4608],
                        vst=vt[:, 0:NCH * 65],
                        vwt=vt[0:112, NCH * 65:VW], ps=ps,
                        pw=pw_tiles[p % 2])

        def pv_step(cx, i):
            # i in 0..31: per q-quarter qg: 4 stripe MMs then 4 window MMs.
            # O'^T accumulates in a [65, 512] quarter; V_aug ones col lands
            # the softmax denominator L in row 64.
            qg, r = i // 8, i % 8
            if r == 0:
                cx["ov" + str(qg)] = o_pool.tile([128, 512], f32, tag="o",
                                                 name=f"ov{cx['p']}_{qg}")
            ov = cx["ov" + str(qg)]
            if r < 4:
                c = r
                nc.tensor.matmul(
                    out=ov[0:65, :],
                    lhsT=cx["vst"][:, c * 65:(c + 1) * 65],
                    rhs=cx["ps"][:, c, qg * 512:(qg + 1) * 512],
                    start=(c == 0), stop=False, skip_group_check=True)
            else:
                j = 4 * qg + (r - 4)
                q0 = (j % 4) * 128
                nc.tensor.matmul(
                    out=ov[0:65, q0:q0 + 128],
                    lhsT=cx["vwt"][:, j * 65:(j + 1) * 65],
                    rhs=cx["pw"][0:112, 128 * j:128 * j + 128],
                    start=False, stop=(r == 7), skip_group_check=True)

        def pv_copy(cx, qg):
            # PSUM -> SBUF staging (DMA cannot read PSUM) on ACT (the
            # exp work lives mostly on DVE), fp32 -> bf16 in the copy;
            # out-DMA per half on two queues
            if qg == 0:
                cx["ob"] = p_pool.tile([65, S], dt_in, tag="ob",
                                       name=f"ob{cx['p']}")
            ob = cx["ob"]
            ov = cx["ov" + str(qg)]
            eng = nc.scalar if qg < 2 else nc.vector
            if eng is nc.scalar:
                eng.copy(ob[:, qg * 512:(qg + 1) * 512], ov[0:65, :])
            else:
                eng.tensor_copy(ob[:, qg * 512:(qg + 1) * 512], ov[0:65, :])
            if qg == 1:
                nc.sync.dma_start(out=out[cx["p"], :, 0:1024],
                                  in_=ob[:, 0:1024])
            if qg == 3:
                nc.gpsimd.dma_start(out=out[cx["p"], :, 1024:2048],
                                    in_=ob[:, 1024:2048])

        # PE clock warmup: the HAM throttles the PE array to half clock
        # until it has seen a few microseconds of sustained matmul
        # activity. Burn junk matmuls (one weight load, pure streaming)
        # into a scratch PSUM bank while the first DMAs are in flight so
        # the real QK matmuls start at the full 2.4 GHz clock.
        warm_sb = v_pool.tile([128, 576], dt_in, tag="warm", name="warm_sb")
        nc.gpsimd.memset(warm_sb, 0.0)
        warm_ps = o_pool.tile([128, 512], f32, tag="o", name="warm_ps")
        for _ in range(8):
            nc.tensor.matmul(out=warm_ps[0:64, :], lhsT=warm_sb[:, 0:64],
                             rhs=warm_sb[:, 64:576], start=True, stop=True,
                             skip_group_check=True)

        ctxs = [load_tiles(0)]
        for pwb in pw_tiles:
            nc.gpsimd.memset(pwb, 0.0)
        for p in range(npairs):
            nxt_needed = p + 1 < npairs
            cur = ctxs[p]
            prev = ctxs[p - 1] if p > 0 else None
            # emit, prefetching next pair's tiles after the first round
            qt, kt, ps, pw = cur["qt"], cur["kt"], cur["ps"], cur["pw"]
            for r in range(8):
                c, g = r % 4, r // 4
                st = s_pool.tile([128, 1024], f32, tag="s")
                for u in range(2):
                    q0 = g * 1024 + u * 512
                    nc.tensor.matmul(
                        out=st[:, u * 512:(u + 1) * 512],
                        lhsT=kt[:, c * 128:(c + 1) * 128],
                        rhs=qt[:, q0:q0 + 512],
                        start=True, stop=True)
                po = ps[:, c, g * 1024:(g + 1) * 1024]
                # drain split tuned for fp32-PSUM costs: ACT is the
                # cheaper PSUM reader (997 vs 1192 ns/tile) and also
                # exact, so it takes 5 of 8 stripe tiles.
                if c <= 1 or (c == 2 and g == 0):
                    nc.scalar.activation(out=po, in_=st,
                                         func=EXP, scale=SCALE)
                else:
                    nc.vector.tensor_scalar(
                        out=po.bitcast(i16), in0=st,
                        scalar1=SCH_A, scalar2=SCH_B, op0=MUL, op1=ADD)
                if r == 0 and nxt_needed:
                    ctxs.append(load_tiles(p + 1))
                if prev is not None:
                    for i in range(4 * r, 4 * r + 4):
                        pv_step(prev, i)
                    if r % 2 == 1:
                        pv_copy(prev, r // 2)
            for h in range(2):
                sw = s_pool.tile([128, 1024], f32, tag="s")
                for j in range(8 * h, 8 * h + 8):
                    fo = (j - 8 * h) * 128
                    nc.tensor.matmul(
                        out=sw[0:112, fo:fo + 128],
                        lhsT=cur["ktw"][:, 128 * j:128 * j + 112],
                        rhs=qt[:, 128 * j:128 * j + 128],
                        start=True, stop=True)
                sw4 = sw.rearrange("p (j t f) -> p j t f", t=2, f=64)
                pw4 = pw.rearrange("p (j t f) -> p j t f", t=2, f=64)
                nc.vector.tensor_scalar(
                    out=pw4[0:48, 8 * h:8 * h + 8, 0, :].bitcast(i16),
                    in0=sw4[0:48, 0:8, 0, :],
                    scalar1=SCH_A, scalar2=SCH_B, op0=MUL, op1=ADD)
                if h == 0:
                    nc.scalar.activation(
                        out=pw4[64:112, 8 * h:8 * h + 8, 1, :],
                        in_=sw4[64:112, 0:8, 1, :],
                        func=EXP, scale=SCALE)
                else:
                    nc.vector.tensor_scalar(
                        out=pw4[64:112, 8 * h:8 * h + 8, 1, :].bitcast(i16),
                        in0=sw4[64:112, 0:8, 1, :],
                        scalar1=SCH_A, scalar2=SCH_B, op0=MUL, op1=ADD)
        # flush: PV of the last pair
        last = ctxs[-1]
        for i in range(32):
            pv_step(last, i)
            if i % 8 == 7:
                pv_copy(last, i // 8)

    nc.compile()
    return nc


def _get_nc(dt_in_name="float16"):
    if dt_in_name not in _CACHE:
        _CACHE[dt_in_name] = _build(dt_in_name)
    return _CACHE[dt_in_name]


def _prep_inputs(query, key, value, np_dt):
    q = np.asarray(query).reshape(NPAIRS, S, D)
    k = np.asarray(key).reshape(NPAIRS, S, D)
    v = np.asarray(value).reshape(NPAIRS, S, D)
    kr = k[:, _REORDER, :]
    vr = v[:, _REORDER, :]
    # qkT: [stripe K^T 512 | Q^T 2048 | window K^T padded: pairs of
    # 128 cols as [48 even | 16 zero | 48 odd | 16 zero]]
    KW = 512 + 64 * NW
    qkT = np.zeros((NPAIRS, 64, S + KW), np_dt)
    kTs = kr.transpose(0, 2, 1).astype(np_dt)  # [P, 64, 2048]
    qkT[:, :, 0:512] = kTs[:, :, 0:512]
    qkT[:, :, 512:512 + S] = q.transpose(0, 2, 1)
    kw = kTs[:, :, 512:].reshape(NPAIRS, 64, NW // 2, 2, 48)
    kTw = qkT[:, :, 512 + S:].reshape(NPAIRS, 64, NW // 2, 2, 64)
    kTw[:, :, :, :, 0:48] = kw
    va = np.concatenate(
        [vr, np.ones((NPAIRS, S, 1), vr.dtype)], axis=2).astype(np_dt)
    # vin: stripe V_aug [partition r, chunk c, 65] ++ window-pair V_aug
    # [112 rows (0:48 window 2i, 64:112 window 2i+1, 48:64 zero), i, 65]
    VW = NCH * 65 + (NW // 2) * 65
    vin = np.zeros((NPAIRS, 128, VW), np_dt)
    vin[:, :, 0:NCH * 65] = (
        va[:, :512].reshape(NPAIRS, NCH, 128, 65).transpose(0, 2, 1, 3)
    ).reshape(NPAIRS, 128, NCH * 65)
    vw = va[:, 512:].reshape(NPAIRS, NW // 2, 2, 48, 65)
    vwin = vin[:, :, NCH * 65:].reshape(NPAIRS, 128, NW // 2, 65)
    vwin[:, 0:48] = vw[:, :, 0].transpose(0, 2, 1, 3)
    vwin[:, 64:112] = vw[:, :, 1].transpose(0, 2, 1, 3)
    in_maps = []
    for core in range(NCORES):
        sl = slice(core * P_PER_CORE, (core + 1) * P_PER_CORE)
        in_maps.append({"qkT": np.ascontiguousarray(qkT[sl]),
                        "vin": np.ascontiguousarray(vin[sl])})
    return in_maps


def _np_dt(dt_in_name):
    if dt_in_name == "float16":
        return np.float16
    if dt_in_name == "bfloat16":
        import ml_dtypes
        return ml_dtypes.bfloat16
    return np.float32


def _run(query, key, value, dt_in_name="float16", trace=False):
    from concourse.bass_utils import run_bass_kernel_spmd
    nc = _get_nc(dt_in_name)
    in_maps = _prep_inputs(query, key, value, _np_dt(dt_in_name))
    res = run_bass_kernel_spmd(nc, in_maps, list(range(NCORES)), trace=trace)
    o = np.concatenate([res.results[i]["out"] for i in range(NCORES)],
                       axis=0).astype(np.float32)
    full = (o[:, 0:64, :] / o[:, 64:65, :]).transpose(0, 2, 1).reshape(
        B, H, S, D).astype(np.float32)
    return full, res


def kernel(query, key, value):
    full, _ = _run(np.asarray(query), np.asarray(key), np.asarray(value))
    return full



# revision 16
# speedup vs baseline: 1.4986x; 1.1752x over previous
"""Block-sparse self-attention (DeepSpeed "fixed" layout) on 8 trn2 cores.

Problem: B=2, H=16, S=2048, D=64 fp32. Mask (identical for every head,
numverts=1): each 64-wide diagonal window is dense, plus every 4th
16-col block ("stripe") is attended by all queries. Per 64-row query
window the attended key set = 512 stripe cols + 48 non-stripe window
cols.

Sharding: 32 (b,h) pairs -> 4 per core (batch+head parallel).

All QK matmuls have contraction K = head_dim = 64, so they run 2-way
row-tiled: the 128x128 PE array is split into two 64-row tiles (T0 on
SBUF partitions 0:64, T8 on 64:128) that stream independent rhs
concurrently, halving QK wall time. Q^T is loaded into both partition
halves; stripe K^T chunks 0,1 / window-pair tiles j=0..7 sit on the
low half (T0), chunks 2,3 / tiles j=8..15 on the high half (T8).

Host prep per pair (pure layout + dtype cast; reorder puts the 512
stripe cols first, then 32 windows x 48 non-stripe cols):
  kq  [128, 1280] row-tiled K^T: [0:64,0:256] stripe chunks 0,1;
      [64:128,0:256] chunks 2,3; cols 256:1280 window-pair tiles of
      128 cols each [48 even | 16 zero | 48 odd | 16 zero] (j 0..7
      low half, j 8..15 high half)
  qd  [64, 2048]  Q^T (DMA'd into both SBUF partition halves)
  vin [128, 4*65 + 16*65] stripe V_aug + window-pair V_aug with a
      ones column each (softmax denominator L rides the PV matmul)

On chip per pair (bf16 operands; PE clock warmed by a junk-matmul
burst during the DMA prologue):
  stripe scores  2 concurrent [128,512] MMs per (chunk-pair, q-half)
  window scores  T0 computes window-pairs j 0..7 (q half 0), T8
                 j 8..15 (q half 1), block-diag exp-write into pw
  P = exp(0.125*s): ACT exact exp on ~half the tiles, DVE
                 Schraudolph-in-bf16-bits on the rest; both engines
                 also split the O' PSUM->SBUF evacuation
  O'^T[65,q] accumulates stripe chunks ([65,512] x16) and window
                 pairs ([65,128] x16) in full-array mode; row 64 = L
Host: O = (O'[0:64] / O'[64])^T per pair.
"""

import numpy as np

B, H, S, D = 2, 16, 2048, 64
NPAIRS = B * H
NCORES = 8
P_PER_CORE = NPAIRS // NCORES  # 4
NCH = 4        # stripe k-chunks of 128
NW = S // 64   # 32 windows
SCALE = float(D) ** -0.5
VW = NCH * 65 + (NW // 2) * 65  # 260 + 1040


def _reorder_idx():
    blocks = np.arange(S // 16)
    stripe = blocks[blocks % 4 == 3]
    rest = blocks[blocks % 4 != 3]
    cols = np.arange(S).reshape(-1, 16)
    return np.concatenate([cols[stripe].ravel(), cols[rest].ravel()])


_REORDER = _reorder_idx()

_CACHE = {}


def _build(dt_in_name="bfloat16", npairs=P_PER_CORE):
    from contextlib import ExitStack
    import concourse.bacc as bacc
    import concourse.tile as tile
    from concourse import mybir

    dt_in = getattr(mybir.dt, dt_in_name)
    f32 = mybir.dt.float32
    i16 = mybir.dt.int16
    EXP = mybir.ActivationFunctionType.Exp
    MUL = mybir.AluOpType.mult
    ADD = mybir.AluOpType.add
    # Schraudolph exp in fp16/bf16 bit space: bits(exp(s*SCALE)) ~
    # s * (SCALE*2^m*log2 e) + (bias*2^m - 0.0579*2^m). One DVE
    # tensor_scalar (fp32 PSUM -> int16 convert) per tile; the int16
    # buffer is the 16-bit-float P tile by bitcast. ~1.5% rms
    # elementwise.
    if dt_in_name == "float16":
        SCH_A = SCALE * 1024.0 / float(np.log(2.0))
        SCH_B = 15.0 * 1024.0 - 59.3
    else:  # bfloat16
        SCH_A = SCALE * 128.0 / float(np.log(2.0))
        SCH_B = 127.0 * 128.0 - 7.4

    nc = bacc.Bacc("TRN2", target_bir_lowering=False, debug=False,
                   num_devices=NCORES)
    kq = nc.dram_tensor("kq", [npairs, 128, 1280], dt_in,
                        kind="ExternalInput").ap()
    qd = nc.dram_tensor("qd", [npairs, 64, S], dt_in,
                        kind="ExternalInput").ap()
    vin = nc.dram_tensor("vin", [npairs, 128, VW], dt_in,
                         kind="ExternalInput").ap()
    out = nc.dram_tensor("out", [npairs, 65, S], dt_in,
                         kind="ExternalOutput").ap()

    with tile.TileContext(nc) as tc, ExitStack() as ctx:
        qk_pool = ctx.enter_context(tc.tile_pool(name="qk", bufs=2))
        v_pool = ctx.enter_context(tc.tile_pool(name="v", bufs=3))
        p_pool = ctx.enter_context(tc.tile_pool(name="p", bufs=2))
        s_pool = ctx.enter_context(tc.tile_pool(name="s", bufs=3, space="PSUM"))
        o_pool = ctx.enter_context(tc.tile_pool(name="o", bufs=2, space="PSUM"))

        # P-window tiles are persistent: the zero cross-window blocks are
        # zeroed once and stay zero (every pair overwrites only the same
        # diagonal blocks), so no per-pair memset is needed.
        pw_tiles = [p_pool.tile([112, S], dt_in, tag=f"pw{b}",
                                name=f"pw_{b}") for b in range(2)]

        # PE clock warmup: the HAM throttles the PE array to half clock
        # until it has seen a few microseconds of sustained matmul
        # activity. Burn junk matmuls (one weight load, pure streaming)
        # into a scratch PSUM bank while the first DMAs are in flight so
        # the real QK matmuls start at the full 2.4 GHz clock.
        warm_sb = v_pool.tile([128, 576], dt_in, tag="warm", name="warm_sb")
        nc.gpsimd.memset(warm_sb, 0.0)
        warm_ps = o_pool.tile([128, 512], f32, tag="o", name="warm_ps")
        for _ in range(8):
            nc.tensor.matmul(out=warm_ps[0:64, :], lhsT=warm_sb[:, 0:64],
                             rhs=warm_sb[:, 64:576], start=True, stop=True,
                             skip_group_check=True)

        def load_tiles(p):
            kw = qk_pool.tile([128, 1280], dt_in, tag="kw",
                              name=f"kw{p}")
            q2 = qk_pool.tile([128, S], dt_in, tag="q2", name=f"q2_{p}")
            if p == 0:
                # split so the first stripe slot's inputs (stripe K +
                # both q halves of q-half 0) land first, across both
                # HWDGE rings.
                nc.sync.dma_start(out=kw[:, 0:256], in_=kq[p, :, 0:256])
                nc.sync.dma_start(out=q2[0:64, 0:1024],
                                  in_=qd[p, :, 0:1024])
                nc.scalar.dma_start(out=q2[64:128, 0:1024],
                                    in_=qd[p, :, 0:1024])
                nc.scalar.dma_start(out=q2[0:64, 1024:2048],
                                    in_=qd[p, :, 1024:2048])
                nc.scalar.dma_start(out=q2[64:128, 1024:2048],
                                    in_=qd[p, :, 1024:2048])
                nc.sync.dma_start(out=kw[:, 256:1280],
                                  in_=kq[p, :, 256:1280])
            else:
                nc.sync.dma_start(out=kw, in_=kq[p])
                nc.gpsimd.dma_start(out=q2[0:64, :], in_=qd[p])
                nc.gpsimd.dma_start(out=q2[64:128, :], in_=qd[p])
            vt = v_pool.tile([128, VW], dt_in, tag="v", name=f"vt{p}")
            nc.gpsimd.dma_start(out=vt, in_=vin[p])
            ps = p_pool.tile([128, NCH, S], dt_in, tag="ps",
                             name=f"ps{p}")
            return dict(p=p, kw=kw, q2=q2,
                        vst=vt[:, 0:NCH * 65],
                        vwt=vt[0:112, NCH * 65:VW], ps=ps,
                        pw=pw_tiles[p % 2])

        def qk_stripe(cur, g, emit_load):
            # Two 64-row PE tiles run concurrently: T0 streams q (low
            # partition copy) against stripe chunks 0,1; T8 streams the
            # high copy against chunks 2,3.
            p = cur["p"]
            kw, q2, ps = cur["kw"], cur["q2"], cur["ps"]
            for cp in range(2):
                stA = s_pool.tile([128, 1024], f32, tag="s",
                                  name=f"sA{p}_{g}{cp}")
                stB = s_pool.tile([128, 1024], f32, tag="s",
                                  name=f"sB{p}_{g}{cp}")
                for u in range(2):
                    q0 = g * 1024 + u * 512
                    nc.tensor.matmul(
                        out=stA[:, u * 512:(u + 1) * 512],
                        lhsT=kw[0:64, cp * 128:(cp + 1) * 128],
                        rhs=q2[0:64, q0:q0 + 512],
                        start=True, stop=True, tile_position=(0, 0))
                    nc.tensor.matmul(
                        out=stB[:, u * 512:(u + 1) * 512],
                        lhsT=kw[64:128, cp * 128:(cp + 1) * 128],
                        rhs=q2[64:128, q0:q0 + 512],
                        start=True, stop=True, tile_position=(64, 0))
                poA = ps[:, cp, g * 1024:(g + 1) * 1024]
                poB = ps[:, 2 + cp, g * 1024:(g + 1) * 1024]
                # drain split tuned for fp32-PSUM costs: ACT is the
                # cheaper PSUM reader (997 vs 1192 ns/tile) and exact,
                # so it takes 5 of the 8 stripe tiles.
                nc.scalar.activation(out=poA, in_=stA, func=EXP,
                                     scale=SCALE)
                if g == 0 and cp == 0:
                    nc.scalar.activation(out=poB, in_=stB, func=EXP,
                                         scale=SCALE)
                else:
                    nc.vector.tensor_scalar(
                        out=poB.bitcast(i16), in0=stB,
                        scalar1=SCH_A, scalar2=SCH_B, op0=MUL, op1=ADD)
                if emit_load and cp == 0:
                    ctxs.append(load_tiles(p + 1))

        def qk_windows(cur):
            p = cur["p"]
            kw, q2, pw = cur["kw"], cur["q2"], cur["pw"]
            swA = s_pool.tile([128, 1024], f32, tag="s", name=f"swA{p}")
            swB = s_pool.tile([128, 1024], f32, tag="s", name=f"swB{p}")
            for jj in range(8):
                # full-128-col weights (the 16-col zero pads ride along)
                # keep Fast Weight Load eligible; rows 112:128 of the
                # scores are zero-garbage and never drained.
                nc.tensor.matmul(
                    out=swA[:, jj * 128:(jj + 1) * 128],
                    lhsT=kw[0:64, 256 + 128 * jj:256 + 128 * (jj + 1)],
                    rhs=q2[0:64, jj * 128:(jj + 1) * 128],
                    start=True, stop=True, tile_position=(0, 0))
                nc.tensor.matmul(
                    out=swB[:, jj * 128:(jj + 1) * 128],
                    lhsT=kw[64:128, 256 + 128 * jj:256 + 128 * (jj + 1)],
                    rhs=q2[64:128, 1024 + jj * 128:1024 + (jj + 1) * 128],
                    start=True, stop=True, tile_position=(64, 0))
            pw4 = pw.rearrange("p (j t f) -> p j t f", t=2, f=64)
            for h, sw in ((0, swA), (1, swB)):
                sw4 = sw.rearrange("p (j t f) -> p j t f", t=2, f=64)
                nc.vector.tensor_scalar(
                    out=pw4[0:48, 8 * h:8 * h + 8, 0, :].bitcast(i16),
                    in0=sw4[0:48, 0:8, 0, :],
                    scalar1=SCH_A, scalar2=SCH_B, op0=MUL, op1=ADD)
                if h == 0:
                    nc.scalar.activation(
                        out=pw4[64:112, 0:8, 1, :],
                        in_=sw4[64:112, 0:8, 1, :],
                        func=EXP, scale=SCALE)
                else:
                    nc.vector.tensor_scalar(
                        out=pw4[64:112, 8:16, 1, :].bitcast(i16),
                        in0=sw4[64:112, 0:8, 1, :],
                        scalar1=SCH_A, scalar2=SCH_B, op0=MUL, op1=ADD)

        def pv_step(cx, i):
            # i in 0..31: per q-quarter qg: 4 stripe MMs then 4 window MMs.
            # O'^T accumulates in a [65, 512] quarter; V_aug ones col lands
            # the softmax denominator L in row 64. Full-array mode.
            qg, r = i // 8, i % 8
            if r == 0:
                cx["ov" + str(qg)] = o_pool.tile([128, 512], f32, tag="o",
                                                 name=f"ov{cx['p']}_{qg}")
            ov = cx["ov" + str(qg)]
            if r < 4:
                c = r
                nc.tensor.matmul(
                    out=ov[0:65, :],
                    lhsT=cx["vst"][:, c * 65:(c + 1) * 65],
                    rhs=cx["ps"][:, c, qg * 512:(qg + 1) * 512],
                    start=(c == 0), stop=False, skip_group_check=True)
            else:
                j = 4 * qg + (r - 4)
                q0 = (j % 4) * 128
                nc.tensor.matmul(
                    out=ov[0:65, q0:q0 + 128],
                    lhsT=cx["vwt"][:, j * 65:(j + 1) * 65],
                    rhs=cx["pw"][0:112, 128 * j:128 * j + 128],
                    start=False, stop=(r == 7), skip_group_check=True)

        def pv_copy(cx, qg):
            # PSUM -> SBUF staging (DMA cannot read PSUM), split over
            # ACT and DVE, fp32 -> bf16 in the copy; out-DMA per half
            # on two queues.
            if qg == 0:
                cx["ob"] = p_pool.tile([65, S], dt_in, tag="ob",
                                       name=f"ob{cx['p']}")
            ob = cx["ob"]
            ov = cx["ov" + str(qg)]
            if qg < 2:
                nc.scalar.copy(ob[:, qg * 512:(qg + 1) * 512], ov[0:65, :])
            else:
                nc.vector.tensor_copy(ob[:, qg * 512:(qg + 1) * 512],
                                      ov[0:65, :])
            if qg == 1:
                nc.sync.dma_start(out=out[cx["p"], :, 0:1024],
                                  in_=ob[:, 0:1024])
            if qg == 3:
                nc.gpsimd.dma_start(out=out[cx["p"], :, 1024:2048],
                                    in_=ob[:, 1024:2048])

        ctxs = [load_tiles(0)]
        for pwb in pw_tiles:
            nc.gpsimd.memset(pwb, 0.0)
        for p in range(npairs):
            cur = ctxs[p]
            prev = ctxs[p - 1] if p > 0 else None
            for g in range(2):
                qk_stripe(cur, g, emit_load=(g == 0 and p + 1 < npairs))
            qk_windows(cur)
            if prev is not None:
                for i in range(32):
                    pv_step(prev, i)
                    if i % 8 == 7:
                        pv_copy(prev, i // 8)
        # flush: PV of the last pair
        last = ctxs[-1]
        for i in range(32):
            pv_step(last, i)
            if i % 8 == 7:
                pv_copy(last, i // 8)

    nc.compile()
    return nc


def _get_nc(dt_in_name="bfloat16"):
    if dt_in_name not in _CACHE:
        _CACHE[dt_in_name] = _build(dt_in_name)
    return _CACHE[dt_in_name]


def _prep_inputs(query, key, value, np_dt):
    q = np.asarray(query).reshape(NPAIRS, S, D)
    k = np.asarray(key).reshape(NPAIRS, S, D)
    v = np.asarray(value).reshape(NPAIRS, S, D)
    kr = k[:, _REORDER, :]
    vr = v[:, _REORDER, :]
    kTs = kr.transpose(0, 2, 1).astype(np_dt)  # [P, 64, 2048]
    # kq row-tiled K^T layout (see _build docstring)
    kq = np.zeros((NPAIRS, 128, 1280), np_dt)
    kq[:, 0:64, 0:256] = kTs[:, :, 0:256]
    kq[:, 64:128, 0:256] = kTs[:, :, 256:512]
    kwp = kTs[:, :, 512:].reshape(NPAIRS, 64, NW // 2, 2, 48)
    ktw = np.zeros((NPAIRS, 64, NW // 2, 2, 64), np_dt)
    ktw[:, :, :, :, 0:48] = kwp
    ktw = ktw.reshape(NPAIRS, 64, S)
    kq[:, 0:64, 256:1280] = ktw[:, :, 0:1024]
    kq[:, 64:128, 256:1280] = ktw[:, :, 1024:2048]
    qdh = np.ascontiguousarray(q.transpose(0, 2, 1).astype(np_dt))
    va = np.concatenate(
        [vr, np.ones((NPAIRS, S, 1), vr.dtype)], axis=2).astype(np_dt)
    # vin: stripe V_aug [partition r, chunk c, 65] ++ window-pair V_aug
    # [112 rows (0:48 window 2i, 64:112 window 2i+1, 48:64 zero), i, 65]
    vin = np.zeros((NPAIRS, 128, VW), np_dt)
    vin[:, :, 0:NCH * 65] = (
        va[:, :512].reshape(NPAIRS, NCH, 128, 65).transpose(0, 2, 1, 3)
    ).reshape(NPAIRS, 128, NCH * 65)
    vw = va[:, 512:].reshape(NPAIRS, NW // 2, 2, 48, 65)
    vwin = vin[:, :, NCH * 65:].reshape(NPAIRS, 128, NW // 2, 65)
    vwin[:, 0:48] = vw[:, :, 0].transpose(0, 2, 1, 3)
    vwin[:, 64:112] = vw[:, :, 1].transpose(0, 2, 1, 3)
    in_maps = []
    for core in range(NCORES):
        sl = slice(core * P_PER_CORE, (core + 1) * P_PER_CORE)
        in_maps.append({"kq": np.ascontiguousarray(kq[sl]),
                        "qd": np.ascontiguousarray(qdh[sl]),
                        "vin": np.ascontiguousarray(vin[sl])})
    return in_maps


def _np_dt(dt_in_name):
    if dt_in_name == "float16":
        return np.float16
    if dt_in_name == "bfloat16":
        import ml_dtypes
        return ml_dtypes.bfloat16
    return np.float32


def _run(query, key, value, dt_in_name="bfloat16", trace=False):
    from concourse.bass_utils import run_bass_kernel_spmd
    nc = _get_nc(dt_in_name)
    in_maps = _prep_inputs(query, key, value, _np_dt(dt_in_name))
    res = run_bass_kernel_spmd(nc, in_maps, list(range(NCORES)), trace=trace)
    o = np.concatenate([res.results[i]["out"] for i in range(NCORES)],
                       axis=0).astype(np.float32)
    full = (o[:, 0:64, :] / o[:, 64:65, :]).transpose(0, 2, 1).reshape(
        B, H, S, D).astype(np.float32)
    return full, res


def kernel(query, key, value):
    full, _ = _run(np.asarray(query), np.asarray(key), np.asarray(value))
    return full
